# revision 4
# baseline (speedup 1.0000x reference)
"""Trainium2 Bass kernel for nn_ASLRNN3 (self-contained).

Math (validated vs the jax reference):
  reference returns (outs[-1], hidden_final). outs[-1] depends only on the
  LAST frame of hand_data plus the hidden recurrence, and the recurrence
  h <- h @ h2h_w.T + h2h_b is input-independent. So:
    - device computes the per-frame branch for frame T-1 only (512 items)
    - host computes the recurrence (exactly; h0 is zeros per the harness
      spec, making it a 32-step row-vector iteration)
    - out = relu(q_dev + (l2_b + h_T) @ out_w.T + out_b), where q_dev =
      i2h_pre @ out_w.T is linear in i2h, so channel-sharded partial i2h
      can be pushed through out_w.T per-core and summed on host.

Device decomposition (all feature-major: features on partitions, all 512
items on the matmul free dim):
  conv1+pool2: pooled p_T = relu(max(A_A.T@x_T, A_B.T@x_T) + bias)
  fc:          both_T windows = relu(Wfc.T @ p_T + bias), emitted as 6
               overlapping row-windows per hand, aligned to conv2 taps
  conv2:       z = sum_{d,k} w2[c,d,k] * shift_k(both_d)  via scaled
               shifted-identity stationary matmuls
  pool3:       max(z, shift1(z), shift2(z)) with shifts as identity
               matmuls, then relu(+conv2 bias)
  l2:          i2h_part_T = l2w_slice @ y2_T   (per-core channel slice)
  out:         q = out_w_fm.T @ i2h_part_T  ->  (10, 512) fp32 per core

Sharding: 8 cores, core j owns conv2 channels [4j, 4j+4). The per-item
hand stage is replicated (cheap); conv2/l2 are channel-sharded.
"""

import numpy as np

# ---------------------------------------------------------------- constants
T, B = 32, 512
HID, OUT = 500, 10
N = B  # items streamed on the free dim
NCORES = 8
CPC = 4  # channels per core

P0S = (0, 126, 252)
WOUTS = (126, 126, 45)
WINS = (128, 128, 47)
C1LO = (0, 128, 256)
C1W = (128, 128, 48)
# fc windows (k, qb) in this order; row offsets k + P0S[qb]
W6 = ((0, 0), (1, 0), (0, 1), (1, 1), (0, 2), (1, 2))
W6LO = tuple(k + P0S[qb] for (k, qb) in W6)
W6W = tuple(WINS[qb] for (k, qb) in W6)
W6OFF = (0, 128, 256, 384, 512, 559)  # cumsum of widths
WFC_STRIDE = 606
MBLK = 4  # i2h split into 4 x 125 rows

_prog = None  # cached (nc, meta)


# ---------------------------------------------------------------- host packs
def _build_A_matrices(cw):
    """A_A/A_B for one hand: A[m, f] with m=x-feature (2j+d), f=o*19+i."""
    A_A = np.zeros((42, 304), np.float32)
    A_B = np.zeros((42, 304), np.float32)
    for o in range(16):
        for i in range(19):
            f = o * 19 + i
            for d in range(2):
                for kk in range(2):
                    A_A[2 * (i + kk) + d, f] += cw[o, d, kk]
                    A_B[2 * (i + 1 + kk) + d, f] += cw[o, d, kk]
    return A_A, A_B


def _host_pack(inputs):
    import ml_dtypes

    bf16 = ml_dtypes.bfloat16
    f32 = np.float32
    x = np.asarray(inputs["hand_data"], f32)[-1]  # (512, 84)
    xl = np.ascontiguousarray(x[:, :42].T).astype(bf16)  # (42, 512)
    xr = np.ascontiguousarray(x[:, 42:].T).astype(bf16)

    aal, abl = _build_A_matrices(np.asarray(inputs["l_conv_w"], f32))
    aar, abr = _build_A_matrices(np.asarray(inputs["r_conv_w"], f32))

    c1b = np.zeros((128, 6), f32)
    for h, cb in enumerate((inputs["l_conv_b"], inputs["r_conv_b"])):
        full = np.repeat(np.asarray(cb, f32), 19)  # (304,) bias per pooled feat
        for t in range(3):
            c1b[: C1W[t], h * 3 + t] = full[C1LO[t] : C1LO[t] + C1W[t]]

    wfc = np.zeros((128, 2 * 3 * WFC_STRIDE), f32)
    fcb = np.zeros((128, 12), f32)
    for h, (fw, fb) in enumerate(
        ((inputs["l_fc_w"], inputs["l_fc_b"]), (inputs["r_fc_w"], inputs["r_fc_b"]))
    ):
        fw = np.asarray(fw, f32)  # (300, 304): [fout, fin]
        fb = np.asarray(fb, f32)
        for t in range(3):
            for w in range(6):
                off = (h * 3 + t) * WFC_STRIDE + W6OFF[w]
                blk = fw[W6LO[w] : W6LO[w] + W6W[w], C1LO[t] : C1LO[t] + C1W[t]].T
                wfc[: C1W[t], off : off + W6W[w]] = blk
        for w in range(6):
            fcb[: W6W[w], h * 6 + w] = fb[W6LO[w] : W6LO[w] + W6W[w]]

    ipad = np.zeros((128, 130), f32)
    for r in range(128):
        ipad[r, r + 2] = 1.0

    w2 = np.asarray(inputs["conv2_w"], f32)  # (32, 2, 2)
    b2 = np.asarray(inputs["conv2_b"], f32)
    l2_w = np.asarray(inputs["l2_w"], f32).reshape(HID, 32, 297)
    out_w = np.asarray(inputs["out_w"], f32)  # (10, 500)

    outw = np.zeros((125, 40), f32)
    for m in range(MBLK):
        outw[:, m * 10 : (m + 1) * 10] = out_w[:, m * 125 : (m + 1) * 125].T

    shared = {
        "xl": xl, "xr": xr,
        "aal": aal.astype(bf16), "abl": abl.astype(bf16),
        "aar": aar.astype(bf16), "abr": abr.astype(bf16),
        "c1b": c1b, "wfc": wfc.astype(bf16), "fcb": fcb,
        "ipad": ipad.astype(bf16), "outw": outw.astype(bf16),
    }

    in_maps = []
    for core in range(NCORES):
        w2b = np.zeros((128, 16), f32)
        b2c = np.zeros((128, CPC), f32)
        l2w = np.zeros((128, 12, HID), f32)
        for c4 in range(CPC):
            ch = core * CPC + c4
            for d in range(2):
                for kk in range(2):
                    w2b[:, c4 * 4 + d * 2 + kk] = w2[ch, d, kk]
            b2c[:, c4] = b2[ch]
            for qb in range(3):
                wo, p0 = WOUTS[qb], P0S[qb]
                l2w[:wo, c4 * 3 + qb, :] = l2_w[:, ch, p0 : p0 + wo].T
        m = dict(shared)
        m["w2b"] = w2b
        m["b2c"] = b2c
        m["l2w"] = l2w.astype(bf16)
        in_maps.append(m)
    return in_maps


def _host_const_and_hidden(inputs):
    f32 = np.float32
    hidden = np.asarray(inputs["hidden"], f32)
    W = np.asarray(inputs["h2h_w"], f32).T
    b = np.asarray(inputs["h2h_b"], f32)
    if np.any(hidden):
        h = hidden.copy()
        for _ in range(T):
            h = h @ W + b
        h_T = h
    else:
        s = np.zeros((HID,), f32)
        for _ in range(T):
            s = s @ W + b
        h_T = np.broadcast_to(s, (B, HID)).copy()
    const = (np.asarray(inputs["l2_b"], f32) + h_T) @ np.asarray(
        inputs["out_w"], f32
    ).T + np.asarray(inputs["out_b"], f32)
    return const, h_T


# ---------------------------------------------------------------- device prog
def _build_program():
    import concourse.mybir as mybir
    from concourse import bacc
    from concourse.tile import TileContext

    F32 = mybir.dt.float32
    BF16 = mybir.dt.bfloat16
    MAX = mybir.AluOpType.max
    ADD = mybir.AluOpType.add
    MULT = mybir.AluOpType.mult
    RELU = mybir.ActivationFunctionType.Relu
    COPY = mybir.ActivationFunctionType.Copy

    nc = bacc.Bacc("TRN2", target_bir_lowering=False, debug=False,
                   num_devices=NCORES)

    def din(name, shape, dt):
        return nc.dram_tensor(name, shape, dt, kind="ExternalInput").ap()

    xl_d = din("xl", [42, N], BF16)
    xr_d = din("xr", [42, N], BF16)
    aal_d = din("aal", [42, 304], BF16)
    abl_d = din("abl", [42, 304], BF16)
    aar_d = din("aar", [42, 304], BF16)
    abr_d = din("abr", [42, 304], BF16)
    c1b_d = din("c1b", [128, 6], F32)
    wfc_d = din("wfc", [128, 2 * 3 * WFC_STRIDE], BF16)
    fcb_d = din("fcb", [128, 12], F32)
    ipad_d = din("ipad", [128, 130], BF16)
    w2b_d = din("w2b", [128, 16], F32)
    b2c_d = din("b2c", [128, CPC], F32)
    l2w_d = din("l2w", [128, 12, HID], BF16)
    outw_d = din("outw", [125, 40], BF16)
    q_d = nc.dram_tensor("q", [OUT, N], F32, kind="ExternalOutput").ap()

    with TileContext(nc) as tc:
        with (
            tc.tile_pool(name="wp", bufs=1) as wp,       # persistent weights
            tc.tile_pool(name="act", bufs=1) as act,     # persistent activations
            tc.tile_pool(name="rot", bufs=4) as rot,     # rotating small sbuf
        ):
            # ---- weight loads
            xl = wp.tile([42, N], BF16); nc.sync.dma_start(out=xl, in_=xl_d)
            xr = wp.tile([42, N], BF16); nc.sync.dma_start(out=xr, in_=xr_d)
            aal = wp.tile([42, 304], BF16); nc.sync.dma_start(out=aal, in_=aal_d)
            abl = wp.tile([42, 304], BF16); nc.sync.dma_start(out=abl, in_=abl_d)
            aar = wp.tile([42, 304], BF16); nc.sync.dma_start(out=aar, in_=aar_d)
            abr = wp.tile([42, 304], BF16); nc.sync.dma_start(out=abr, in_=abr_d)
            c1b = wp.tile([128, 6], F32); nc.sync.dma_start(out=c1b, in_=c1b_d)
            wfc = wp.tile([128, 2 * 3 * WFC_STRIDE], BF16)
            nc.sync.dma_start(out=wfc, in_=wfc_d)
            fcb = wp.tile([128, 12], F32); nc.sync.dma_start(out=fcb, in_=fcb_d)
            ipad = wp.tile([128, 130], BF16); nc.sync.dma_start(out=ipad, in_=ipad_d)
            w2b = wp.tile([128, 16], F32); nc.sync.dma_start(out=w2b, in_=w2b_d)
            b2c = wp.tile([128, CPC], F32); nc.sync.dma_start(out=b2c, in_=b2c_d)
            l2w = wp.tile([128, 12, HID], BF16); nc.sync.dma_start(out=l2w, in_=l2w_d)
            outw = wp.tile([125, 40], BF16); nc.sync.dma_start(out=outw, in_=outw_d)

            # ---- scaled shifted identities: sIp[:, i*130:(i+1)*130]
            sIp = act.tile([128, 16 * 130], BF16)
            for i in range(16):
                nc.vector.tensor_scalar(
                    out=sIp[:, i * 130 : (i + 1) * 130], in0=ipad,
                    scalar1=w2b[:, i : i + 1], scalar2=None, op0=MULT,
                )

            # ---- conv1 + pool2 + relu  ->  p_sb[h] tile [128, 3, N]
            p_sb = [None, None]
            with tc.tile_pool(name="ps_c1", bufs=1, space="PSUM") as ps_c1:
                for h, (xh, aa, ab) in enumerate(((xl, aal, abl), (xr, aar, abr))):
                    pt = act.tile([128, 3, N], BF16, tag=f"p_sb{h}",
                                  name=f"p_sb{h}")
                    p_sb[h] = pt
                    for t in range(3):
                        lo, w = C1LO[t], C1W[t]
                        pa = ps_c1.tile([128, N], F32, name="pa", bufs=2)
                        pb = ps_c1.tile([128, N], F32, name="pb", bufs=2)
                        nc.tensor.matmul(pa[:w], aa[:, lo : lo + w], xh,
                                         start=True, stop=True)
                        nc.tensor.matmul(pb[:w], ab[:, lo : lo + w], xh,
                                         start=True, stop=True)
                        pa_sb = rot.tile([128, N], BF16, tag="pa_sb",
                                         name="pa_sb")
                        nc.scalar.activation(pa_sb[:w], pa[:w], COPY)
                        u = rot.tile([128, N], BF16, tag="u_c1", name="u_c1")
                        nc.vector.tensor_tensor(out=u[:w], in0=pa_sb[:w],
                                                in1=pb[:w], op=MAX)
                        nc.vector.tensor_scalar(
                            out=pt[:w, t], in0=u[:w],
                            scalar1=c1b[:w, h * 3 + t : h * 3 + t + 1],
                            scalar2=0.0, op0=ADD, op1=MAX,
                        )

            # ---- fc -> both_sb[h][w] (6 overlapping windows per hand)
            both_sb = [[None] * 6 for _ in range(2)]
            with tc.tile_pool(name="ps_fc", bufs=1, space="PSUM") as ps_fc:
                for h in range(2):
                    for w in range(6):
                        ww, off = W6W[w], W6OFF[w]
                        fcp = ps_fc.tile([128, N], F32, name="fcp", bufs=3)
                        for t in range(3):
                            base = (h * 3 + t) * WFC_STRIDE + off
                            nc.tensor.matmul(
                                fcp[:ww], wfc[: C1W[t], base : base + ww],
                                p_sb[h][: C1W[t], t],
                                start=(t == 0), stop=(t == 2),
                            )
                        bt = act.tile([128, N], BF16, tag=f"both{h}_{w}")
                        both_sb[h][w] = bt
                        nc.scalar.activation(
                            bt[:ww], fcp[:ww], RELU,
                            bias=fcb[:ww, h * 6 + w : h * 6 + w + 1],
                        )

            # ---- conv2 + pool3 -> y2_sb[12]
            y2_sb = [None] * 12
            with (
                tc.tile_pool(name="ps_z", bufs=1, space="PSUM") as ps_z,
                tc.tile_pool(name="ps_sh", bufs=1, space="PSUM") as ps_sh,
            ):
                for c4 in range(CPC):
                    for qb in range(3):
                        wi, wo = WINS[qb], WOUTS[qb]
                        widx = {0: W6.index((0, qb)), 1: W6.index((1, qb))}
                        za = ps_z.tile([128, N], F32, name="za", bufs=2)
                        for d in range(2):
                            for kk in range(2):
                                i = c4 * 4 + d * 2 + kk
                                nc.tensor.matmul(
                                    za[:wi],
                                    sIp[:wi, i * 130 + 2 : i * 130 + 2 + wi],
                                    both_sb[d][widx[kk]][:wi],
                                    start=(d == 0 and kk == 0),
                                    stop=(d == 1 and kk == 1),
                                )
                        z_sb = rot.tile([128, N], BF16, tag="z_sb")
                        nc.scalar.activation(z_sb[:wi], za[:wi], COPY)
                        zb = ps_sh.tile([128, N], F32, name="zb", bufs=2)
                        zc = ps_sh.tile([128, N], F32, name="zc", bufs=2)
                        nc.tensor.matmul(zb[:wo], ipad[:wi, 3 : 3 + wo],
                                         z_sb[:wi], start=True, stop=True)
                        nc.tensor.matmul(zc[:wo], ipad[:wi, 4 : 4 + wo],
                                         z_sb[:wi], start=True, stop=True)
                        t1 = rot.tile([128, N], BF16, tag="t1")
                        nc.vector.tensor_tensor(out=t1[:wo], in0=z_sb[:wo],
                                                in1=zb[:wo], op=MAX)
                        u2 = rot.tile([128, N], BF16, tag="u2")
                        nc.vector.tensor_tensor(out=u2[:wo], in0=t1[:wo],
                                                in1=zc[:wo], op=MAX)
                        yt = act.tile([128, N], BF16, tag=f"y2_{c4}_{qb}")
                        y2_sb[c4 * 3 + qb] = yt
                        nc.vector.tensor_scalar(
                            out=yt[:wo], in0=u2[:wo],
                            scalar1=b2c[:wo, c4 : c4 + 1],
                            scalar2=0.0, op0=ADD, op1=MAX,
                        )

            # ---- l2 (48 accumulating matmuls) + i2h evac + out matmul
            with tc.tile_pool(name="ps_l2", bufs=1, space="PSUM") as ps_l2:
                i2h_ps = [ps_l2.tile([125, N], F32, tag=f"i2h{m}", name=f"i2h{m}")
                          for m in range(MBLK)]
                nkt = CPC * 3
                for m in range(MBLK):
                    for kt in range(nkt):
                        wo = WOUTS[kt % 3]
                        nc.tensor.matmul(
                            i2h_ps[m],
                            l2w[:wo, kt, m * 125 : (m + 1) * 125],
                            y2_sb[kt][:wo],
                            start=(kt == 0), stop=(kt == nkt - 1),
                        )
                qp = ps_l2.tile([OUT, N], F32, tag="qp")
                for m in range(MBLK):
                    ih = act.tile([125, N], BF16, tag="ih", bufs=2, name="ih")
                    nc.scalar.activation(ih, i2h_ps[m], COPY)
                    nc.tensor.matmul(qp, outw[:, m * 10 : (m + 1) * 10], ih,
                                     start=(m == 0), stop=(m == MBLK - 1))
                q_sb = act.tile([OUT, N], F32, tag="q_sb")
                nc.vector.tensor_copy(out=q_sb, in_=qp)
                nc.sync.dma_start(out=q_d, in_=q_sb)

    nc.compile()
    return nc


def _get_program():
    global _prog
    if _prog is None:
        _prog = _build_program()
    return _prog


# ---------------------------------------------------------------- entry point
def kernel(**inputs):
    from concourse.bass_utils import run_bass_kernel_spmd

    nc = _get_program()
    in_maps = _host_pack(inputs)
    res = run_bass_kernel_spmd(nc, in_maps, core_ids=list(range(NCORES)))
    q_total = np.zeros((N, OUT), np.float32)
    for c in range(NCORES):
        q_total += res.results[c]["q"].T
    const, h_T = _host_const_and_hidden(inputs)
    out = np.maximum(q_total + const, 0.0).astype(np.float32)
    return out, h_T.astype(np.float32)
